# revision 23
# baseline (speedup 1.0000x reference)
"""Exponentiated-quadratic (RBF) kernel matrix on 8 Trainium2 NeuronCores.

K[i, j] = sigma * exp(-0.5 * ||x1_i/rho - x2_j/rho||^2)
        with sigma = exp(log_sigma)^2, rho = exp(log_rho)

Strategy
--------
Row-shard x1 across the 8 cores (512 rows each), replicate x2. The squared
distance folds into a single augmented matmul: with
  a_i = [x1_i/rho, -0.5*||x1_i/rho||^2, 1]          (P+2 = 34 wide)
  b_j = [x2_j/rho, 1, -0.5*||x2_j/rho||^2]
the dot product a_i . b_j = -0.5 * d_ij, so the whole epilogue is one
ScalarE activation: K = exp(s + 2*log_sigma)  (ACT computes func(x*scale+bias)
for free). Host prep is only the tiny (N,P) scaling/transpose; all O(N*M)
work (matmul, exp, output traffic) runs on-device.

The matmul runs as a 3-pass bf16 split (s = Ah.Bh + Ah.Bl + Al.Bh with
A = Ah + Al exactly in bf16) accumulated in fp32 PSUM: bf16 streams the PE
at 1 column/cycle (fp32 is 4x slower, fp32r is TF32-precision) and the
dropped Al.Bl term is ~2^-18 relative — measured 1.9e-5 scale-relative
output error.

Per core: 4 row-blocks of 128 x 4096. Each row-block is 2 PSUM tiles
(128 x 2048 = 4 banks); each PSUM tile takes 12 matmuls (3 passes x 4
column-slices of 512), one exp-activation PSUM->SBUF, and the row-block
ends with one 2 MiB DMA to HBM. Output-DMA roofline: 8 MiB/core at
~360 GB/s ~= 23 us.

walrus in this container rejects instructions carrying more than one
semaphore wait, which shapes three things: the first B chunk is fused into
the same DRAM tensor/DMA as A (first matmul = one wait), PSUM tiles are
persistent (pool re-allocation adds a same-engine PE wait), and a chain of
single-wait NOPs on the sync sequencer "observes" every completion before
the framework's kernel-tail drain (which otherwise waits on all ~10 sems
at once).
"""

import numpy as np
import ml_dtypes

import concourse.bass as bass
import concourse.mybir as mybir
import concourse.tile as tile
from concourse.bass_utils import run_bass_kernel_spmd
from concourse.tile import add_dep_helper

N, M, P = 4096, 4096, 32
NCORES = 8
NSHARD = N // NCORES  # 512 rows of x1 per core
KAUG = P + 2          # 34: contraction dim after augmentation
IBLK = 128            # output row-block = PSUM partition dim
JBLK = 512            # matmul free dim = one fp32 PSUM bank
PSW = 2048            # PSUM tile width (4 banks) = one exp-activation
BCH = 2048            # B chunk width (pipelines input load under compute)

BF16 = mybir.dt.bfloat16
NPBF16 = ml_dtypes.bfloat16


def _build_nc(bias_val: float):
    nc = bass.Bass()
    # l1 = [A_hi | A_lo | B_hi[:, :BCH]] fused into one DMA so the first
    # matmul depends on a single semaphore.
    l1_w = 2 * NSHARD + BCH
    l1_t = nc.declare_dram_parameter("l1_t", [KAUG, l1_w], BF16, isOutput=False)
    bl0_t = nc.declare_dram_parameter("bl0_t", [KAUG, BCH], BF16, isOutput=False)
    bh1_t = nc.declare_dram_parameter("bh1_t", [KAUG, M - BCH], BF16, isOutput=False)
    bl1_t = nc.declare_dram_parameter("bl1_t", [KAUG, M - BCH], BF16, isOutput=False)
    out = nc.declare_dram_parameter("out", [NSHARD, M], mybir.dt.float32, isOutput=True)

    with tile.TileContext(nc) as tc:
        with (
            tc.tile_pool(name="inp", bufs=1) as inp_pool,
            tc.tile_pool(name="stage", bufs=1) as stage_pool,
            tc.tile_pool(name="ps", bufs=1, space="PSUM") as ps_pool,
        ):
            dma_insts = []
            l1_sb = inp_pool.tile([KAUG, l1_w], BF16, tag="l1")
            dma_insts.append(nc.sync.dma_start(out=l1_sb, in_=l1_t[:, :]))
            bl0_sb = inp_pool.tile([KAUG, BCH], BF16, tag="bl0")
            dma_insts.append(nc.sync.dma_start(out=bl0_sb, in_=bl0_t[:, :]))
            bh1_sb = inp_pool.tile([KAUG, M - BCH], BF16, tag="bh1")
            dma_insts.append(nc.sync.dma_start(out=bh1_sb, in_=bh1_t[:, :]))
            bl1_sb = inp_pool.tile([KAUG, M - BCH], BF16, tag="bl1")
            dma_insts.append(nc.sync.dma_start(out=bl1_sb, in_=bl1_t[:, :]))

            a_hi = l1_sb[:, 0:NSHARD]
            a_lo = l1_sb[:, NSHARD : 2 * NSHARD]
            bh_chunks = [l1_sb[:, 2 * NSHARD : 2 * NSHARD + BCH], bh1_sb[:, :]]
            bl_chunks = [bl0_sb[:, :], bl1_sb[:, :]]

            # Persistent PSUM tiles (see module docstring).
            ps_tiles = [
                ps_pool.tile(
                    [IBLK, PSW], mybir.dt.float32, tag=f"ps{h}", name=f"ps{h}"
                )
                for h in range(M // PSW)
            ]

            act_insts = []
            mm_insts = []
            for i in range(NSHARD // IBLK):  # 4 row-blocks
                out_sb = stage_pool.tile(
                    [IBLK, M], mybir.dt.float32, tag=f"out{i}", name=f"out{i}"
                )
                lhs_hi = a_hi[:, i * IBLK : (i + 1) * IBLK]
                lhs_lo = a_lo[:, i * IBLK : (i + 1) * IBLK]
                for h in range(M // PSW):  # 2 PSUM tiles per row-block
                    ps = ps_tiles[h]
                    bh_ch = bh_chunks[h]
                    bl_ch = bl_chunks[h]
                    nq = PSW // JBLK  # 4 column slices
                    # 3 passes; same stationary weights within a pass so the
                    # PE's LDWEIGHTS prefetch stays hot.
                    for lhsT, rhs_ch, start, stop in (
                        (lhs_hi, bh_ch, True, False),
                        (lhs_hi, bl_ch, False, False),
                        (lhs_lo, bh_ch, False, True),
                    ):
                        for q in range(nq):
                            mm_insts.append(
                                nc.tensor.matmul(
                                    ps[:, q * JBLK : (q + 1) * JBLK],
                                    lhsT=lhsT,
                                    rhs=rhs_ch[:, q * JBLK : (q + 1) * JBLK],
                                    start=start,
                                    stop=stop,
                                )
                            )
                    act_insts.append(
                        nc.scalar.activation(
                            out=out_sb[:, h * PSW : (h + 1) * PSW],
                            in_=ps,
                            func=mybir.ActivationFunctionType.Exp,
                            bias=float(bias_val),
                            scale=1.0,
                        )
                    )
                dma_insts.append(
                    nc.sync.dma_start(
                        out=out[i * IBLK : (i + 1) * IBLK, :], in_=out_sb
                    )
                )

            # ACT->ACT sync deps come from PSUM bank-pair serialization of two
            # READS of the same tile — already transitively ordered through
            # the interleaved matmuls, and same-engine FIFO besides. Demote to
            # nosync (ordering-only): walrus rejects ACTIVATE instructions
            # carrying more than one semaphore wait.
            import bass_rust as _br

            act_names = {a.ins.name for a in act_insts}
            for a in act_insts:
                deps = list(a.ins.sync_dependency_names())
                spurious = [d for d in deps if d in act_names]
                if spurious:
                    keep = [d for d in deps if d not in act_names]
                    a.ins.take_sync_dependencies()
                    a.ins.set_sync_dependencies(
                        _br.InstructionNameOrderedSet(keep)
                    )
                    a.ins.add_nosync_dependencies_from(
                        _br.InstructionNameOrderedSet(spurious)
                    )

            # Wait-funnel for the kernel-tail drain: the framework drain waits
            # on every live semaphore at once, which walrus rejects (sync-wait
            # slot limit). Observe each completion on the SP sequencer via
            # single-wait nops first, so the drain itself needs no waits.
            for t in [mm_insts[-1], act_insts[-1], *dma_insts]:
                nop = nc.sync.nop(nofuse=True, hint="tail_funnel")
                add_dep_helper(nop.ins, t.ins, True, "tail wait funnel")
    return nc


def run(x1, x2, log_rho, log_sigma, trace=False):
    """Returns (K, exec_time_ns). exec_time_ns is None unless trace=True."""
    x1 = np.asarray(x1, dtype=np.float32)
    x2 = np.asarray(x2, dtype=np.float32)
    rho = float(np.exp(np.float64(np.asarray(log_rho))))
    bias = 2.0 * float(np.asarray(log_sigma))  # log(sigma) = 2*log_sigma

    xs = (x1 / np.float32(rho)).astype(np.float32)
    ys = (x2 / np.float32(rho)).astype(np.float32)
    xn = np.einsum("np,np->n", xs, xs, dtype=np.float64)
    yn = np.einsum("mp,mp->m", ys, ys, dtype=np.float64)

    a_full = np.empty((KAUG, N), np.float32)
    a_full[:P] = xs.T
    a_full[P] = (-0.5 * xn).astype(np.float32)
    a_full[P + 1] = 1.0
    b_full = np.empty((KAUG, M), np.float32)
    b_full[:P] = ys.T
    b_full[P] = 1.0
    b_full[P + 1] = (-0.5 * yn).astype(np.float32)

    a_hi = a_full.astype(NPBF16)
    a_lo = (a_full - a_hi.astype(np.float32)).astype(NPBF16)
    b_hi = b_full.astype(NPBF16)
    b_lo = (b_full - b_hi.astype(np.float32)).astype(NPBF16)

    bl0 = np.ascontiguousarray(b_lo[:, :BCH])
    bh1 = np.ascontiguousarray(b_hi[:, BCH:])
    bl1 = np.ascontiguousarray(b_lo[:, BCH:])

    nc = _build_nc(bias)
    in_maps = []
    for c in range(NCORES):
        sl = slice(c * NSHARD, (c + 1) * NSHARD)
        l1 = np.concatenate([a_hi[:, sl], a_lo[:, sl], b_hi[:, :BCH]], axis=1)
        in_maps.append(
            {
                "l1_t": np.ascontiguousarray(l1),
                "bl0_t": bl0,
                "bh1_t": bh1,
                "bl1_t": bl1,
            }
        )
    res = run_bass_kernel_spmd(
        nc, in_maps, core_ids=list(range(NCORES)), trace=trace
    )
    full = np.concatenate(
        [res.results[c]["out"] for c in range(NCORES)], axis=0
    )
    return full, res.exec_time_ns


def kernel(x1, x2, log_rho, log_sigma):
    out, _ = run(x1, x2, log_rho, log_sigma, trace=False)
    return out


# revision 30
# speedup vs baseline: 1.0560x; 1.0560x over previous
"""Exponentiated-quadratic (RBF) kernel matrix on 8 Trainium2 NeuronCores.

K[i, j] = sigma * exp(-0.5 * ||x1_i/rho - x2_j/rho||^2)
        with sigma = exp(log_sigma)^2, rho = exp(log_rho)

Strategy
--------
Row-shard x1 across the 8 cores (512 rows each), replicate x2. The squared
distance folds into a single augmented matmul: with
  a_i = [x1_i/rho, -0.5*||x1_i/rho||^2, 1]          (P+2 = 34 wide)
  b_j = [x2_j/rho, 1, -0.5*||x2_j/rho||^2]
the dot product a_i . b_j = -0.5 * d_ij, so the whole epilogue is one
ScalarE activation: K = exp(s + 2*log_sigma)  (ACT computes func(x*scale+bias)
for free). Host prep is only the tiny (N,P) scaling/transpose; all O(N*M)
work (matmul, exp, output traffic) runs on-device.

The matmul runs as a 3-pass bf16 split (s = Ah.Bh + Ah.Bl + Al.Bh with
A = Ah + Al exactly in bf16) accumulated in fp32 PSUM: bf16 streams the PE
at 1 column/cycle (fp32 is 4x slower, fp32r is TF32-precision) and the
dropped Al.Bl term is ~2^-18 relative — measured 1.9e-5 scale-relative
output error.

Per core: 4 row-blocks of 128 x 4096. Each row-block is 2 PSUM tiles
(128 x 2048 = 4 banks); each PSUM tile takes 12 matmuls (3 passes x 4
column-slices of 512), one exp-activation PSUM->SBUF, and the row-block
ends with one 2 MiB DMA to HBM. Output-DMA roofline: 8 MiB/core at
~360 GB/s ~= 23 us.

walrus in this container rejects instructions carrying more than one
semaphore wait, which shapes three things: the first B chunk is fused into
the same DRAM tensor/DMA as A (first matmul = one wait), PSUM tiles are
persistent (pool re-allocation adds a same-engine PE wait), and a chain of
single-wait NOPs on the sync sequencer "observes" every completion before
the framework's kernel-tail drain (which otherwise waits on all ~10 sems
at once).
"""

import numpy as np
import ml_dtypes

import concourse.bass as bass
import concourse.mybir as mybir
import concourse.tile as tile
from concourse.bass_utils import run_bass_kernel_spmd
from concourse.tile import add_dep_helper

N, M, P = 4096, 4096, 32
NCORES = 8
NSHARD = N // NCORES  # 512 rows of x1 per core
KAUG = P + 2          # 34: contraction dim after augmentation
IBLK = 128            # output row-block = PSUM partition dim
JBLK = 512            # matmul free dim = one fp32 PSUM bank
PSW = 2048            # PSUM tile width (4 banks) = one exp-activation
BCH = 2048            # B chunk width (pipelines input load under compute)

BF16 = mybir.dt.bfloat16
NPBF16 = ml_dtypes.bfloat16
STRIP = 64            # PE row-strip stride for 2-way row packing (K=34 -> 64)


def _build_nc(bias_val: float):
    nc = bass.Bass()
    # in1 = [A_hi | A_lo | B_hi[:, :BCH] | B_lo[:, :BCH]]: one DMA per input
    # tensor so each matmul depends on a single semaphore, and total DMA
    # count stays at 8 (one per HWDGE lane — lane reuse adds an ordering
    # wait that walrus rejects).
    in1_w = 2 * NSHARD + 2 * BCH
    in2_w = 2 * (M - BCH)
    in1_t = nc.declare_dram_parameter("in1_t", [KAUG, in1_w], BF16, isOutput=False)
    in2_t = nc.declare_dram_parameter("in2_t", [KAUG, in2_w], BF16, isOutput=False)
    out = nc.declare_dram_parameter("out", [NSHARD, M], mybir.dt.float32, isOutput=True)

    with tile.TileContext(nc) as tc:
        with (
            tc.tile_pool(name="inp", bufs=1) as inp_pool,
            tc.tile_pool(name="stage", bufs=1) as stage_pool,
            tc.tile_pool(name="ps", bufs=1, space="PSUM") as ps_pool,
        ):
            # Inputs land in partitions 0..33 (row strip 0), then an
            # SBUF->SBUF DMA replicates them to partitions 64..97 (row strip
            # 1). Matmuls alternate strips via tile_position so two streams
            # run concurrently in the PE array and each strip's LDWEIGHTS
            # prefetches under the other strip's matmul.
            dma_insts = []
            in1_sb = inp_pool.tile([2 * STRIP, in1_w], BF16, tag="in1")
            in2_sb = inp_pool.tile([2 * STRIP, in2_w], BF16, tag="in2")
            for sb, dram in ((in1_sb, in1_t), (in2_sb, in2_t)):
                dma_insts.append(
                    nc.sync.dma_start(out=sb[:KAUG, :], in_=dram[:, :])
                )
                dma_insts.append(
                    nc.sync.dma_start(
                        out=sb[STRIP : STRIP + KAUG, :], in_=sb[:KAUG, :]
                    )
                )

            def strip_aps(s):
                r = slice(s * STRIP, s * STRIP + KAUG)
                return (
                    in1_sb[r, 0:NSHARD],                      # a_hi
                    in1_sb[r, NSHARD : 2 * NSHARD],           # a_lo
                    [
                        in1_sb[r, 2 * NSHARD : 2 * NSHARD + BCH],
                        in2_sb[r, 0 : M - BCH],
                    ],
                    [
                        in1_sb[r, 2 * NSHARD + BCH : 2 * NSHARD + 2 * BCH],
                        in2_sb[r, M - BCH : 2 * (M - BCH)],
                    ],
                )

            strips = [strip_aps(0), strip_aps(1)]

            # Persistent PSUM tiles (see module docstring).
            ps_tiles = [
                ps_pool.tile(
                    [IBLK, PSW], mybir.dt.float32, tag=f"ps{h}", name=f"ps{h}"
                )
                for h in range(M // PSW)
            ]

            act_insts = []
            mm_insts = []
            for i in range(NSHARD // IBLK):  # 4 row-blocks
                out_sb = stage_pool.tile(
                    [IBLK, M], mybir.dt.float32, tag=f"out{i}", name=f"out{i}"
                )
                for h in range(M // PSW):  # 2 PSUM tiles per row-block
                    ps = ps_tiles[h]
                    nq = PSW // JBLK  # 4 column slices
                    # 3 passes; column slice q runs in row strip q%2.
                    for pass_idx, (start, stop) in enumerate(
                        ((True, False), (False, False), (False, True))
                    ):
                        for q in range(nq):
                            s = q % 2
                            a_hi, a_lo, bh_chunks, bl_chunks = strips[s]
                            lhsT = (a_hi, a_hi, a_lo)[pass_idx][
                                :, i * IBLK : (i + 1) * IBLK
                            ]
                            rhs_ch = (bh_chunks, bl_chunks, bh_chunks)[
                                pass_idx
                            ][h]
                            mm_insts.append(
                                nc.tensor.matmul(
                                    ps[:, q * JBLK : (q + 1) * JBLK],
                                    lhsT=lhsT,
                                    rhs=rhs_ch[:, q * JBLK : (q + 1) * JBLK],
                                    start=start,
                                    stop=stop,
                                    tile_position=(s * STRIP, 0),
                                )
                            )
                    act_insts.append(
                        nc.scalar.activation(
                            out=out_sb[:, h * PSW : (h + 1) * PSW],
                            in_=ps,
                            func=mybir.ActivationFunctionType.Exp,
                            bias=float(bias_val),
                            scale=1.0,
                        )
                    )
                dma_insts.append(
                    nc.sync.dma_start(
                        out=out[i * IBLK : (i + 1) * IBLK, :], in_=out_sb
                    )
                )

            # ACT->ACT sync deps come from PSUM bank-pair serialization of two
            # READS of the same tile — already transitively ordered through
            # the interleaved matmuls, and same-engine FIFO besides. Demote to
            # nosync (ordering-only): walrus rejects ACTIVATE instructions
            # carrying more than one semaphore wait.
            import bass_rust as _br

            act_names = {a.ins.name for a in act_insts}
            for a in act_insts:
                deps = list(a.ins.sync_dependency_names())
                spurious = [d for d in deps if d in act_names]
                if spurious:
                    keep = [d for d in deps if d not in act_names]
                    a.ins.take_sync_dependencies()
                    a.ins.set_sync_dependencies(
                        _br.InstructionNameOrderedSet(keep)
                    )
                    a.ins.add_nosync_dependencies_from(
                        _br.InstructionNameOrderedSet(spurious)
                    )

            # Wait-funnel for the kernel-tail drain: the framework drain waits
            # on every live semaphore at once, which walrus rejects (sync-wait
            # slot limit). Observe each completion on the SP sequencer via
            # single-wait nops first, so the drain itself needs no waits.
            for t in [mm_insts[-1], act_insts[-1], *dma_insts]:
                nop = nc.sync.nop(nofuse=True, hint="tail_funnel")
                add_dep_helper(nop.ins, t.ins, True, "tail wait funnel")
    return nc


def run(x1, x2, log_rho, log_sigma, trace=False):
    """Returns (K, exec_time_ns). exec_time_ns is None unless trace=True."""
    x1 = np.asarray(x1, dtype=np.float32)
    x2 = np.asarray(x2, dtype=np.float32)
    rho = float(np.exp(np.float64(np.asarray(log_rho))))
    bias = 2.0 * float(np.asarray(log_sigma))  # log(sigma) = 2*log_sigma

    xs = (x1 / np.float32(rho)).astype(np.float32)
    ys = (x2 / np.float32(rho)).astype(np.float32)
    xn = np.einsum("np,np->n", xs, xs, dtype=np.float64)
    yn = np.einsum("mp,mp->m", ys, ys, dtype=np.float64)

    a_full = np.empty((KAUG, N), np.float32)
    a_full[:P] = xs.T
    a_full[P] = (-0.5 * xn).astype(np.float32)
    a_full[P + 1] = 1.0
    b_full = np.empty((KAUG, M), np.float32)
    b_full[:P] = ys.T
    b_full[P] = 1.0
    b_full[P + 1] = (-0.5 * yn).astype(np.float32)

    a_hi = a_full.astype(NPBF16)
    a_lo = (a_full - a_hi.astype(np.float32)).astype(NPBF16)
    b_hi = b_full.astype(NPBF16)
    b_lo = (b_full - b_hi.astype(np.float32)).astype(NPBF16)

    in2 = np.ascontiguousarray(
        np.concatenate([b_hi[:, BCH:], b_lo[:, BCH:]], axis=1)
    )

    nc = _build_nc(bias)
    in_maps = []
    for c in range(NCORES):
        sl = slice(c * NSHARD, (c + 1) * NSHARD)
        in1 = np.concatenate(
            [a_hi[:, sl], a_lo[:, sl], b_hi[:, :BCH], b_lo[:, :BCH]], axis=1
        )
        in_maps.append(
            {"in1_t": np.ascontiguousarray(in1), "in2_t": in2}
        )
    res = run_bass_kernel_spmd(
        nc, in_maps, core_ids=list(range(NCORES)), trace=trace
    )
    full = np.concatenate(
        [res.results[c]["out"] for c in range(NCORES)], axis=0
    )
    return full, res.exec_time_ns


def kernel(x1, x2, log_rho, log_sigma):
    out, _ = run(x1, x2, log_rho, log_sigma, trace=False)
    return out
